# revision 47
# baseline (speedup 1.0000x reference)
"""Trainium2 Bass kernel for a causal self-attention block (nanogpt-style).

Full inputs -> full output. 16 heads sharded 2/core across 8 NeuronCores
(tensor-parallel); each core computes its heads' QKV projection, per-head
RMSNorm + RoPE, causal no-max-softmax attention, and a partial c_proj over
its 128-dim slice of the residual. Host sums the 8 bf16 partial outputs.

v2 interleaved-pipeline rewrite of the phased baseline (300us -> target):
  - single merged pipeline: tile-prep (QKV/norm/rope/transpose) for group
    g+1 is emitted BETWEEN the attention chunk-pairs of q-block g, so the
    ACT engine (exp, the attention pacer) and the PE (QKV, the prep pacer)
    overlap instead of running in separate phases
  - host-side DMA repack: x/w/cos/sin/ve stored partition-major per tile so
    every DMA is a contiguous >=1KB-per-partition burst (the baseline's
    rearranged loads were 32-256B descriptors and stalled the first 20us)
  - rsqrt for RMSNorm via exp(-0.5*ln(x)): Ln and Exp share one ACT table
    (sqrt does not -> table thrash); one table load for the whole kernel
  - PSUM pressure: 3 rings — scr 2x[128,1024] (scores + QKV), po
    2x[128,512] (c_proj halves + sel-broadcast + transposes via bf16
    bitcast view), py 2x[96,512] (PV accum). py is released early by a
    bulk psum->sbuf bf16 eviction (ysb) right after the last PV matmul;
    den/yT work then runs SBUF-side (bf16, 2x DVE modes)
  - tri-mask on the idle Pool engine (latency-tolerant: y lags scores by
    the software pipeline); everything psum-touching stays on DVE (Pool
    cannot access PSUM on TRN2)
  - deferred finalize as in baseline: each q-block's den/sel/yT/c_proj is
    drained inside the NEXT q-block's chunk loop
"""

import numpy as np

DIM = 1024
NH = 16
HD = 64
SCALE = 0.12
NC_CORES = 8
HPC = NH // NC_CORES  # 2 heads per core


def _build(T=4096):
    import concourse.bass as bass
    import concourse.tile as tile
    from concourse import mybir

    f32 = mybir.dt.float32
    bf16 = mybir.dt.bfloat16
    AF = mybir.ActivationFunctionType

    NTT = T // 128   # 32 t-tiles of 128
    NQB = T // 512   # 8 q-blocks of 512
    G = 4            # tiles per norm/rope group
    NG = NTT // G
    EPS = float(np.finfo(np.float32).eps)

    nc = bass.Bass("TRN2", target_bir_lowering=False, debug=False,
                   num_devices=NC_CORES)

    xt4 = nc.declare_dram_parameter("xt4", [128, NTT * 1024], bf16,
                                    isOutput=False).ap()
    wp = nc.declare_dram_parameter("wp", [128, 8 * 384], bf16,
                                   isOutput=False).ap()
    vep = nc.declare_dram_parameter("vep", [128, NTT * 192], bf16,
                                    isOutput=False).ap()
    cosp = nc.declare_dram_parameter("cosp", [128, NTT * 16], bf16,
                                     isOutput=False).ap()
    sinp = nc.declare_dram_parameter("sinp", [128, NTT * 16], bf16,
                                     isOutput=False).ap()
    sel = nc.declare_dram_parameter("sel", [64, 128], bf16, isOutput=False).ap()
    tri = nc.declare_dram_parameter("tri", [128, 128], bf16, isOutput=False).ap()
    iden = nc.declare_dram_parameter("iden", [128, 128], bf16, isOutput=False).ap()
    wcT = nc.declare_dram_parameter("wcT", [128, DIM], bf16, isOutput=False).ap()
    outp = nc.declare_dram_parameter("outp", [T, DIM], bf16, isOutput=True).ap()
    import os
    debug = os.environ.get('KDEBUG', '0') == '1'
    if debug:
        dQT = nc.declare_dram_parameter("dQT", [128, T], bf16, isOutput=True).ap()
        dKT = nc.declare_dram_parameter("dKT", [128, T], bf16, isOutput=True).ap()
        dV = nc.declare_dram_parameter("dV", [128, NTT * 192], bf16, isOutput=True).ap()
        dYT = nc.declare_dram_parameter("dYT", [128, T], bf16, isOutput=True).ap()
        dDEN = nc.declare_dram_parameter("dDEN", [64, T], bf16, isOutput=True).ap()

    with tile.TileContext(nc) as tc:
        with (
            tc.tile_pool(name="consts", bufs=1) as consts,
            tc.tile_pool(name="persist", bufs=1) as persist,
            tc.tile_pool(name="xstream", bufs=5) as xstream,
            tc.tile_pool(name="tmp", bufs=4) as tmp,
            tc.tile_pool(name="pt", bufs=10) as ptpool,
            tc.tile_pool(name="ob", bufs=3) as obpool,
            tc.tile_pool(name="ysb", bufs=4) as ysbpool,
            tc.tile_pool(name="small", bufs=6) as small,
            tc.tile_pool(name="sc", bufs=2, space="PSUM") as scr,   # 4 banks
            tc.tile_pool(name="py", bufs=2, space="PSUM") as pyp,   # 2 banks
            tc.tile_pool(name="po", bufs=2, space="PSUM") as pop,   # 2 banks
        ):
            # ---- w half 1 first (gates the first QKV), then the small
            # consts; x stream starts in parallel via the lookahead below --
            eps_sb = consts.tile([128, 1], f32, tag="eps")
            nc.gpsimd.memset(eps_sb[:, :], EPS)
            w_lo = consts.tile([128, 4, 384], bf16, tag="wlo")
            nc.sync.dma_start(
                w_lo[:, :, :],
                wp[:, 0:1536].rearrange("p (dc e) -> p dc e", dc=4))
            tri_sb = consts.tile([128, 128], bf16, tag="tri")
            nc.gpsimd.dma_start(tri_sb[:, :], tri)
            id_sb = consts.tile([128, 128], bf16, tag="iden")
            nc.gpsimd.dma_start(id_sb[:, :], iden)
            sel_sb = consts.tile([64, 128], bf16, tag="sel")
            nc.gpsimd.dma_start(sel_sb[:, :], sel)

            q_nat = persist.tile([128, NTT, 128], bf16, tag="qnat")
            k_nat = persist.tile([128, NTT, 128], bf16, tag="knat")
            # rider layout: v0 | ones | v1 | ones (denominator rides as
            # redundant ones columns in the M=96 PV matmul)
            v_sb = persist.tile([128, NTT, 192], bf16, tag="v")
            QT = persist.tile([128, T], bf16, tag="QT")
            KT = persist.tile([128, T], bf16, tag="KT")
            yT = persist.tile([128, T], bf16, tag="yT")

            xts = {}
            dma_done = [0]

            def issue_dma(tt):
                if tt >= NTT or tt < dma_done[0]:
                    return
                dma_done[0] = tt + 1
                xt = xstream.tile([128, 8, 128], bf16, tag="xt",
                                  name=f"xt{tt}")
                nc.sync.dma_start(
                    xt[:, 0:4, :],
                    xt4[:, 1024 * tt:1024 * tt + 512]
                    .rearrange("p (dc t) -> p dc t", dc=4))
                nc.sync.dma_start(
                    xt[:, 4:8, :],
                    xt4[:, 1024 * tt + 512:1024 * tt + 1024]
                    .rearrange("p (dc t) -> p dc t", dc=4))
                xts[tt] = xt
                nc.gpsimd.dma_start(v_sb[:, tt, :],
                                    vep[:, 192 * tt:192 * tt + 192])

            issue_dma(0)
            w_hi = consts.tile([128, 4, 384], bf16, tag="whi")
            nc.sync.dma_start(
                w_hi[:, :, :],
                wp[:, 1536:3072].rearrange("p (dc e) -> p dc e", dc=4))
            issue_dma(1)
            issue_dma(2)
            issue_dma(3)
            # remaining consts after the first x tiles, on the Pool and ACT
            # descriptor rings so they don't serialize behind the x stream
            cos_sb = consts.tile([128, NTT, 16], bf16, tag="cos")
            nc.gpsimd.dma_start(cos_sb[:, :, :],
                                cosp.rearrange("p (tt i) -> p tt i", i=16))
            sin_sb = consts.tile([128, NTT, 16], bf16, tag="sin")
            nc.gpsimd.dma_start(sin_sb[:, :, :],
                                sinp.rearrange("p (tt i) -> p tt i", i=16))
            wc_sb = consts.tile([128, DIM], bf16, tag="wc")
            nc.scalar.dma_start(wc_sb[:, :], wcT)

            # ---- work item generators ----
            def w1(tt):
                """QKV projection of tile tt + eviction. Early tiles evict
                q/k via the ACT engine (idle until the exp stream ramps);
                late tiles use DVE (which has slack once prep ends)."""
                def f():
                    issue_dma(tt + 4)
                    xt = xts.pop(tt)
                    ps = scr.tile([128, 1024], f32, tag="sc", name=f"ps1_{tt}")
                    for dc in range(8):
                        wt = w_lo if dc < 4 else w_hi
                        nc.tensor.matmul(ps[:, 0:384], xt[:, dc, :],
                                         wt[:, dc % 4, :],
                                         start=(dc == 0), stop=(dc == 7))
                    cp = nc.scalar.copy if tt < 12 else nc.vector.tensor_copy
                    cp(q_nat[:, tt, :], ps[:, 0:128])
                    cp(k_nat[:, tt, :], ps[:, 128:256])
                    nc.vector.tensor_add(v_sb[:, tt, 0:64], ps[:, 256:320],
                                         v_sb[:, tt, 0:64])
                    nc.vector.tensor_add(v_sb[:, tt, 96:160], ps[:, 320:384],
                                         v_sb[:, tt, 96:160])
                return f

            def w2a(g):
                """sum-of-squares + rsqrt for tiles 4g..4g+3.
                rsqrt(ms+eps) = exp(-0.5*ln(ms+eps)): Ln+Exp share an ACT
                table so there are no table reloads against the softmax
                exp stream."""
                def f():
                    gsl = slice(G * g, G * g + G)
                    ssA = small.tile([128, 2, G, 2], f32, tag="ssA",
                                     name=f"ss{g}")
                    for qki, nat in enumerate((q_nat, k_nat)):
                        xg = nat[:, gsl, :]
                        sq = tmp.tile([128, G * 128], bf16, tag="sq")
                        nc.vector.tensor_mul(sq[:, :], xg, xg)
                        nc.vector.reduce_sum(
                            ssA[:, qki, :, :],
                            sq[:, :].rearrange("p (a h d) -> p a h d",
                                               a=G, h=2),
                            axis=mybir.AxisListType.X)
                    lnt = small.tile([128, 2, G, 2], f32, tag="lnt",
                                     name=f"ln{g}")
                    nc.scalar.activation(lnt[:, :, :, :], ssA[:, :, :, :],
                                         AF.Ln, bias=eps_sb[:, :],
                                         scale=1.0 / HD)
                    rinv = small.tile([128, 2, G, 2], f32, tag="rinv",
                                      name=f"ri{g}")
                    nc.scalar.activation(rinv[:, :, :, :], lnt[:, :, :, :],
                                         AF.Exp, scale=-0.5)
                    rinvb = small.tile([128, 2, G, 2], bf16, tag="rinvb",
                                       name=f"rb{g}")
                    nc.vector.tensor_copy(rinvb[:, :, :, :],
                                          rinv[:, :, :, :])
                    return rinvb
                holder = {}

                def run():
                    holder['rinvb'] = f()
                run.holder = holder
                return run

            def w2b(g, w2a_item, qki):
                """normalize + rope one side (q or k) of group g."""
                def f():
                    rinvb = w2a_item.holder['rinvb']
                    gsl = slice(G * g, G * g + G)
                    nat = (q_nat, k_nat)[qki]
                    xg4 = nat[:, gsl, :].rearrange(
                        "p a (h d) -> p a h d", h=2)
                    nc.vector.tensor_mul(
                        xg4, xg4,
                        rinvb[:, qki, :, :]
                        .rearrange("p a h -> p a h ()")
                        .broadcast_to((128, G, 2, HD)))
                    # rope on pairs (d, d+32), d in [0,16)
                    x1 = xg4[:, :, :, 0:16]
                    x2 = xg4[:, :, :, 32:48]
                    cg = (cos_sb[:, gsl, :].rearrange("p a i -> p a () i")
                          .broadcast_to((128, G, 2, 16)))
                    sg = (sin_sb[:, gsl, :].rearrange("p a i -> p a () i")
                          .broadcast_to((128, G, 2, 16)))
                    t1 = tmp.tile([128, G, 2, 16], bf16, tag="t1")
                    t2 = tmp.tile([128, G, 2, 16], bf16, tag="t2")
                    t3 = tmp.tile([128, G, 2, 16], bf16, tag="t3")
                    t4 = tmp.tile([128, G, 2, 16], bf16, tag="t4")
                    nc.vector.tensor_mul(t1[:, :, :, :], x1, cg)
                    nc.vector.tensor_mul(t2[:, :, :, :], x2, sg)
                    nc.vector.tensor_mul(t3[:, :, :, :], x1, sg)
                    nc.vector.tensor_mul(t4[:, :, :, :], x2, cg)
                    nc.vector.tensor_add(x1, t1[:, :, :, :],
                                         t2[:, :, :, :])
                    nc.vector.tensor_sub(x2, t4[:, :, :, :],
                                         t3[:, :, :, :])
                return f

            def w3(tt):
                """transpose tile tt into QT/KT via a po-ring slot (bf16
                bitcast view; q and k land in the same bank, so the k
                transpose must not re-clear it -> separate slots)."""
                def f():
                    for ni, (nat, dstT) in enumerate(((q_nat, QT),
                                                      (k_nat, KT))):
                        slot = pop.tile([128, 512], f32, tag="po",
                                        name=f"tp{tt}_{ni}")
                        sb = slot[:, 0:64].bitcast(bf16)
                        nc.tensor.transpose(sb, nat[:, tt, :], id_sb[:, :])
                        cp = (nc.scalar.copy if tt < 12
                              else nc.vector.tensor_copy)
                        cp(dstT[:, 128 * tt:128 * tt + 128], sb)
                return f

            # ---- attention emission ----
            def emit_y(py, pts, ntc, head_major=False):
                """PV accumulation with lhsT = [V_h(64) | ones(32)]: M=96,
                the ones columns accumulate the softmax denominator free.
                head_major orders the final flush so head 0 completes while
                head 1's PVs still stream (the den chain starts earlier)."""
                order = ([(h, c, ql, pt) for h in range(2)
                          for (c, ql, pt) in pts] if head_major else
                         [(h, c, ql, pt) for (c, ql, pt) in pts
                          for h in range(2)])
                for (h, c, ql, pt) in order:
                    nc.tensor.matmul(py[h][:, ql:512],
                                     v_sb[:, c, 96 * h:96 * h + 96],
                                     pt[:, h, ql:512],
                                     start=(c == 0), stop=(c == ntc - 1))

            def cproj(tt, qb, holder):
                def f():
                    ts2 = slice(128 * tt, 128 * tt + 128)
                    # per-tile softmax normalize of yT: lets tile tt's
                    # c_proj start as soon as its own 128 columns are
                    # normalized (pipelines the last q-block's tail)
                    lt = 128 * (tt - 4 * qb)
                    for h in range(2):
                        nc.vector.tensor_mul(
                            yT[64 * h:64 * h + 64, ts2],
                            holder['ysb2'][64 * h:64 * h + 64,
                                           lt:lt + 128],
                            holder['rrs'][64 * h:64 * h + 64,
                                          lt:lt + 128])
                    for half in range(2):
                        po = pop.tile([128, 512], f32, tag="po",
                                      name=f"po{tt}_{half}")
                        nc.tensor.matmul(
                            po[:, :], yT[:, ts2],
                            wc_sb[:, 512 * half:512 * half + 512],
                            start=True, stop=True)
                        ob = obpool.tile([128, 512], bf16, tag="ob",
                                         name=f"ob{tt}_{half}")
                        # early tiles land while the exp stream is sparse:
                        # ACT absorbs those casts, DVE keeps the late ones
                        if tt < 12:
                            nc.scalar.copy(ob[:, :], po[:, :])
                        else:
                            nc.vector.tensor_copy(ob[:, :], po[:, :])
                        # all output DMAs on the Sync ring: the Pool
                        # ring's context-exit drain runs first and would
                        # serialize the teardown behind them
                        nc.sync.dma_start(
                            outp[ts2, 512 * half:512 * half + 512],
                            ob[:, :])
                return f

            # ---- prologue: prep groups 0 and 1 (the per-group chain
            # DMA->QKV->evict->norm->rope->transpose has ~10us latency;
            # early q-blocks are shorter than that, so prep runs 2 groups
            # ahead of the q-block that consumes it) ----
            a0 = w2a(0)
            a1 = w2a(1)
            prologue = [w1(0), w1(1), w1(2), w1(3),
                        a0, w1(4), w1(5), w2b(0, a0, 0), w2b(0, a0, 1),
                        w1(6), w1(7), w3(0), w3(1),
                        a1, w3(2), w3(3), w2b(1, a1, 0), w2b(1, a1, 1),
                        w3(4), w3(5), w3(6), w3(7)]
            for f in prologue:
                f()

            pending = []  # deferred finalize + next-group prep closures

            for qb in range(NQB):
                q0 = 512 * qb
                ntc = 4 * (qb + 1)
                npairs = ntc // 2
                # queue next group's prep behind any finalize items already
                # pending from qb-1, interleaved prep-first
                if qb < NQB - 2:
                    g = qb + 2
                    a = w2a(g)
                    prep = [w1(G * g), w1(G * g + 1), w1(G * g + 2),
                            w1(G * g + 3), a, w2b(g, a, 0), w2b(g, a, 1),
                            w3(G * g), w3(G * g + 1), w3(G * g + 2),
                            w3(G * g + 3)]
                    fins = pending
                    items = []
                    while prep or fins:
                        if prep:
                            items.append(prep.pop(0))
                        if prep:
                            items.append(prep.pop(0))
                        if fins:
                            items.append(fins.pop(0))
                    pending = items

                py = [pyp.tile([96, 512], f32, tag="py",
                               name=f"py{qb}_{h}") for h in range(2)]
                stage = []  # software pipeline: y lags scores by 3 pairs
                total = len(pending)
                for pi in range(npairs):
                    pts = []
                    schunks = []
                    # both chunks' score matmuls first (contiguous PE
                    # burst), then both exps: fewer PE pipeline breaks
                    for c in (2 * pi, 2 * pi + 1):
                        p = c - 4 * qb
                        ql = max(0, 128 * p)
                        ts = slice(128 * c, 128 * c + 128)
                        s = scr.tile([128, 1024], f32, tag="sc",
                                     name=f"s{qb}_{c}")
                        for h in range(2):
                            hp = slice(64 * h, 64 * h + 64)
                            nc.tensor.matmul(
                                s[:, 512 * h + ql:512 * h + 512],
                                KT[hp, ts], QT[hp, q0 + ql:q0 + 512],
                                start=True, stop=True,
                                tile_position=(64 * h, 0))
                        schunks.append((c, p, ql, s))
                    for (c, p, ql, s) in schunks:
                        pt = ptpool.tile([128, 2, 512], bf16, tag="pt",
                                         name=f"ptile{qb}_{c}")
                        nc.scalar.activation(
                            pt[:, :, ql:512],
                            s[:, :].rearrange("p (h n) -> p h n",
                                              h=2)[:, :, ql:512],
                            AF.Exp, scale=SCALE)
                        if p >= 0:
                            # causal mask on the diagonal chunk: Pool engine
                            # (latency hidden by the y-stage pipeline)
                            nc.gpsimd.tensor_mul(
                                pt[:, :, ql:ql + 128],
                                pt[:, :, ql:ql + 128],
                                tri_sb[:, :].rearrange("p x -> p () x")
                                .broadcast_to((128, 2, 128)))
                        pts.append((c, ql, pt))
                    stage.append(pts)
                    if len(stage) == 5:
                        emit_y(py, stage.pop(0), ntc)
                    # drain queued prep/finalize proportionally
                    want = (total * (pi + 1)) // npairs
                    while total - len(pending) < want:
                        pending.pop(0)()
                while len(stage) > 1:
                    emit_y(py, stage.pop(0), ntc)
                final_pts = stage.pop(0)
                while pending:
                    pending.pop(0)()

                # ---- final PVs head-major + per-head early py eviction:
                # head h's ysb copy + den transpose start while head h+1's
                # PVs still stream (shortens the serial finalize chain,
                # which is the whole tail on the last q-block) ----
                # ysb2: y rows to SBUF bf16 with heads at rows 64h so the
                # later mult's SBUF operands share base partitions.
                # den: rider rows 64:96 hold den replicated x32; transpose
                # so q spreads across lanes, reciprocal 16 els/lane,
                # transpose back; den2 rows {0,32} = 1/den per head.
                ysb2 = ysbpool.tile([128, 512], bf16, tag="ysb",
                                    name=f"ysb{qb}")
                xss = []
                actden = qb <= 4 or qb == NQB - 1
                if actden:
                    # mid phase + last block: evict both riders into ONE
                    # [64,512] tile (rows 32h = head h); 1/den =
                    # exp(-ln(den)) then takes a single ACT ln + exp pair
                    # in fin_norm, where its input is long-ready so it
                    # can't head-of-line-block the exp stream
                    xs2 = small.tile([64, 512], f32, tag="xs2",
                                     name=f"xs2_{qb}")
                    xss.append(xs2)
                for h in range(2):
                    for (c, ql, pt) in final_pts:
                        nc.tensor.matmul(py[h][:, ql:512],
                                         v_sb[:, c, 96 * h:96 * h + 96],
                                         pt[:, h, ql:512],
                                         start=(c == 0), stop=(c == ntc - 1))
                    nc.vector.tensor_copy(ysb2[64 * h:64 * h + 64, :],
                                          py[h][0:64, :])
                    if actden:
                        nc.vector.tensor_copy(xss[0][32 * h:32 * h + 32, :],
                                              py[h][64:96, :])
                    else:
                        # late phase: ACT is exp-saturated, keep the DVE
                        # stream-transpose reciprocal trick
                        xs = small.tile([32, 512], f32, tag="xs",
                                        name=f"xs{qb}_{h}")
                        nc.vector.transpose(xs[0:32, :], py[h][64:96, :])
                        xss.append(xs)

                def fin_norm(qb=qb, q0=q0, ysb2=ysb2, xss=xss):
                    rdenb = small.tile([64, 512], bf16, tag="rdenb",
                                       name=f"rb{qb}")
                    if qb <= 4 or qb == NQB - 1:
                        lden = small.tile([64, 512], f32, tag="den2",
                                          name=f"ld{qb}")
                        nc.scalar.activation(lden[:, :], xss[0][:, :],
                                             AF.Ln)
                        nc.scalar.activation(rdenb[0:64, :], lden[:, :],
                                             AF.Exp, scale=-1.0)
                    else:
                        den2 = small.tile([64, 512], f32, tag="den2",
                                          name=f"d2{qb}")
                        for h in range(2):
                            xv = xss[h][0:32, :].rearrange(
                                "p (j c) -> p j c", c=32)
                            nc.vector.reciprocal(xv[:, :, 0:1],
                                                 xv[:, :, 0:1])
                            nc.vector.transpose(den2[32 * h:32 * h + 32, :],
                                                xss[h][0:32, :])
                        nc.vector.tensor_copy(rdenb[0:33, :], den2[0:33, :])
                    if debug:
                        nc.sync.dma_start(dDEN[:, q0:q0 + 512],
                                          rdenb[0:64, :])
                    rp = pop.tile([128, 512], f32, tag="po",
                                  name=f"rp{qb}")
                    nc.tensor.matmul(rp[:, :], sel_sb[0:33, :],
                                     rdenb[0:33, :], start=True, stop=True)
                    rrs = tmp.tile([128, 512], bf16, tag="rrs",
                                   name=f"rrs{qb}")
                    nc.vector.tensor_copy(rrs[:, :], rp[:, :])
                    fin_norm.holder['rrs'] = rrs

                fin_norm.holder = {'ysb2': ysb2}
                pending.append(fin_norm)
                for tt in range(4 * qb, 4 * qb + 4):
                    pending.append(cproj(tt, qb, fin_norm.holder))
            while pending:
                pending.pop(0)()
            if debug:
                nc.sync.dma_start(dQT[:, :], QT[:, :])
                nc.sync.dma_start(dKT[:, :], KT[:, :])
                nc.sync.dma_start(
                    dV[:, :], v_sb[:, :, :].rearrange("p tt d -> p (tt d)"))
                nc.sync.dma_start(dYT[:, :], yT[:, :])
    _cap_matmul_waits(nc)
    return nc


def _cap_matmul_waits(nc, limit=1):
    """walrus supports few (often one) sync-wait slots per lowered
    instruction; move excess waits onto same-engine nops inserted just
    before, so the sequencer blocks identically but each instruction
    carries at most `limit` waits."""
    import bass_rust
    from concourse import mybir

    eng = {
        mybir.EngineType.PE: nc.tensor,
        mybir.EngineType.DVE: nc.vector,
        mybir.EngineType.Activation: nc.scalar,
        mybir.EngineType.Pool: nc.gpsimd,
        mybir.EngineType.SP: nc.sync,
    }

    def make_nop(e):
        eng[e].nop()
        fn = nc.m.functions[0]
        for obb in fn.blocks:
            if (obb.instructions
                    and type(obb.instructions[-1]).__name__ == 'InstNoOp'):
                return obb.instructions.pop()
        raise AssertionError('nop not found')

    fn = nc.m.functions[0]
    for bb in fn.blocks:
        il = bb.instructions
        i = 0
        while i < len(il):
            inst = il[i]
            si = inst.sync_info
            if (si and si.on_wait and len(si.on_wait) > limit
                    and inst.engine in eng
                    and type(inst).__name__ != 'InstNoOp'):
                waits = list(si.on_wait)
                keep, excess = waits[-limit:], waits[:-limit]
                for w in excess:
                    nop = make_nop(inst.engine)
                    nop.sync_info = bass_rust.SyncInfo(on_wait=[w],
                                                       on_update=[])
                    il.insert(i, nop)
                    i += 1
                inst.sync_info = bass_rust.SyncInfo(
                    on_wait=keep, on_update=list(si.on_update))
            i += 1


def _host_prep(x, ve, qkv_w, lambdas, c_proj_w, T):
    import ml_dtypes
    bf = ml_dtypes.bfloat16
    NTT = T // 128
    xf = x.reshape(T, DIM)
    # xt4[p, tt, dc, ti] = x[128*tt+ti, 128*dc+p]
    xt4 = np.ascontiguousarray(
        xf.reshape(NTT, 128, 8, 128).transpose(3, 0, 2, 1)
        .reshape(128, NTT * 1024).astype(bf))
    af = (1.0 / 1024.0) ** np.linspace(0.0, 1.0, HD // 4, dtype=np.float32)
    theta = np.arange(T, dtype=np.float32)[:, None] * af[None, :]
    # cosp[p, tt, i] = cos[128*tt+p, i]
    cosp = np.ascontiguousarray(
        np.cos(theta).reshape(NTT, 128, 16).transpose(1, 0, 2)
        .reshape(128, NTT * 16).astype(bf))
    sinp = np.ascontiguousarray(
        np.sin(theta).reshape(NTT, 128, 16).transpose(1, 0, 2)
        .reshape(128, NTT * 16).astype(bf))
    tri = np.ascontiguousarray(np.triu(np.ones((128, 128), np.float32)).astype(bf))
    iden = np.ascontiguousarray(np.eye(128, dtype=np.float32).astype(bf))
    lam = np.asarray(lambdas, np.float32)
    vef = ve.reshape(T, DIM)
    selm = np.zeros((64, 128), np.float32)
    selm[0, 0:64] = 1.0
    selm[32, 64:128] = 1.0
    selm = np.ascontiguousarray(selm.astype(bf))
    in_maps = []
    for c in range(NC_CORES):
        sl = slice(128 * c, 128 * c + 128)
        wq = qkv_w[0][sl]
        wk = qkv_w[1][sl]
        wv = qkv_w[2][sl] * lam[0]
        wTl = np.concatenate([wq, wk, wv], 0).T  # [1024, 384]
        # wp[p, dc, e] = wTl[128*dc+p, e]
        wpl = np.ascontiguousarray(
            wTl.reshape(8, 128, 384).transpose(1, 0, 2)
            .reshape(128, 8 * 384).astype(bf))
        ve_l = np.ones((T, 192), np.float32)
        ve_l[:, 0:64] = vef[:, sl.start:sl.start + 64] * lam[1]
        ve_l[:, 96:160] = vef[:, sl.start + 64:sl.stop] * lam[1]
        # vep[p, tt, d] = ve_l[128*tt+p, d]
        vepl = np.ascontiguousarray(
            ve_l.reshape(NTT, 128, 192).transpose(1, 0, 2)
            .reshape(128, NTT * 192).astype(bf))
        wcTl = np.ascontiguousarray(c_proj_w[:, sl].T.astype(bf))
        in_maps.append(dict(xt4=xt4, wp=wpl, vep=vepl, cosp=cosp, sinp=sinp,
                            tri=tri, iden=iden, wcT=wcTl, sel=selm))
    return in_maps


LAST_RESULTS = None


def kernel(x, ve, qkv_w, lambdas, c_proj_w):
    import sys
    if '/opt/trn_rl_repo' not in sys.path:
        sys.path.insert(0, '/opt/trn_rl_repo')
    from concourse.bass_utils import run_bass_kernel_spmd

    x = np.asarray(x, np.float32)
    T = x.shape[1]
    in_maps = _host_prep(np.asarray(x, np.float32), np.asarray(ve, np.float32),
                         np.asarray(qkv_w, np.float32),
                         np.asarray(lambdas, np.float32),
                         np.asarray(c_proj_w, np.float32), T)
    nc = _build(T)
    res = run_bass_kernel_spmd(nc, in_maps, core_ids=list(range(NC_CORES)))
    global LAST_RESULTS
    LAST_RESULTS = res
    out = np.zeros((T, DIM), np.float32)
    for rmap in res.results:
        out += rmap["outp"].astype(np.float32)
    return out.reshape(1, T, DIM)


# revision 48
# speedup vs baseline: 1.0047x; 1.0047x over previous
"""Trainium2 Bass kernel for a causal self-attention block (nanogpt-style).

Full inputs -> full output. 16 heads sharded 2/core across 8 NeuronCores
(tensor-parallel); each core computes its heads' QKV projection, per-head
RMSNorm + RoPE, causal no-max-softmax attention, and a partial c_proj over
its 128-dim slice of the residual. Host sums the 8 bf16 partial outputs.

v2 interleaved-pipeline rewrite of the phased baseline (300us -> target):
  - single merged pipeline: tile-prep (QKV/norm/rope/transpose) for group
    g+1 is emitted BETWEEN the attention chunk-pairs of q-block g, so the
    ACT engine (exp, the attention pacer) and the PE (QKV, the prep pacer)
    overlap instead of running in separate phases
  - host-side DMA repack: x/w/cos/sin/ve stored partition-major per tile so
    every DMA is a contiguous >=1KB-per-partition burst (the baseline's
    rearranged loads were 32-256B descriptors and stalled the first 20us)
  - rsqrt for RMSNorm via exp(-0.5*ln(x)): Ln and Exp share one ACT table
    (sqrt does not -> table thrash); one table load for the whole kernel
  - PSUM pressure: 3 rings — scr 2x[128,1024] (scores + QKV), po
    2x[128,512] (c_proj halves + sel-broadcast + transposes via bf16
    bitcast view), py 2x[96,512] (PV accum). py is released early by a
    bulk psum->sbuf bf16 eviction (ysb) right after the last PV matmul;
    den/yT work then runs SBUF-side (bf16, 2x DVE modes)
  - tri-mask on the idle Pool engine (latency-tolerant: y lags scores by
    the software pipeline); everything psum-touching stays on DVE (Pool
    cannot access PSUM on TRN2)
  - deferred finalize as in baseline: each q-block's den/sel/yT/c_proj is
    drained inside the NEXT q-block's chunk loop
"""

import numpy as np

DIM = 1024
NH = 16
HD = 64
SCALE = 0.12
NC_CORES = 8
HPC = NH // NC_CORES  # 2 heads per core


def _build(T=4096):
    import concourse.bass as bass
    import concourse.tile as tile
    from concourse import mybir

    f32 = mybir.dt.float32
    bf16 = mybir.dt.bfloat16
    AF = mybir.ActivationFunctionType

    NTT = T // 128   # 32 t-tiles of 128
    NQB = T // 512   # 8 q-blocks of 512
    G = 4            # tiles per norm/rope group
    NG = NTT // G
    EPS = float(np.finfo(np.float32).eps)

    nc = bass.Bass("TRN2", target_bir_lowering=False, debug=False,
                   num_devices=NC_CORES)

    xt4 = nc.declare_dram_parameter("xt4", [128, NTT * 1024], bf16,
                                    isOutput=False).ap()
    wp = nc.declare_dram_parameter("wp", [128, 8 * 384], bf16,
                                   isOutput=False).ap()
    vep = nc.declare_dram_parameter("vep", [128, NTT * 192], bf16,
                                    isOutput=False).ap()
    cosp = nc.declare_dram_parameter("cosp", [128, NTT * 16], bf16,
                                     isOutput=False).ap()
    sinp = nc.declare_dram_parameter("sinp", [128, NTT * 16], bf16,
                                     isOutput=False).ap()
    sel = nc.declare_dram_parameter("sel", [64, 128], bf16, isOutput=False).ap()
    tri = nc.declare_dram_parameter("tri", [128, 128], bf16, isOutput=False).ap()
    iden = nc.declare_dram_parameter("iden", [128, 128], bf16, isOutput=False).ap()
    wcT = nc.declare_dram_parameter("wcT", [128, DIM], bf16, isOutput=False).ap()
    outp = nc.declare_dram_parameter("outp", [T, DIM], bf16, isOutput=True).ap()
    import os
    debug = os.environ.get('KDEBUG', '0') == '1'
    if debug:
        dQT = nc.declare_dram_parameter("dQT", [128, T], bf16, isOutput=True).ap()
        dKT = nc.declare_dram_parameter("dKT", [128, T], bf16, isOutput=True).ap()
        dV = nc.declare_dram_parameter("dV", [128, NTT * 192], bf16, isOutput=True).ap()
        dYT = nc.declare_dram_parameter("dYT", [128, T], bf16, isOutput=True).ap()
        dDEN = nc.declare_dram_parameter("dDEN", [64, T], bf16, isOutput=True).ap()

    with tile.TileContext(nc) as tc:
        with (
            tc.tile_pool(name="consts", bufs=1) as consts,
            tc.tile_pool(name="persist", bufs=1) as persist,
            tc.tile_pool(name="xstream", bufs=5) as xstream,
            tc.tile_pool(name="tmp", bufs=4) as tmp,
            tc.tile_pool(name="pt", bufs=10) as ptpool,
            tc.tile_pool(name="ob", bufs=3) as obpool,
            tc.tile_pool(name="ysb", bufs=4) as ysbpool,
            tc.tile_pool(name="small", bufs=6) as small,
            tc.tile_pool(name="sc", bufs=2, space="PSUM") as scr,   # 4 banks
            tc.tile_pool(name="py", bufs=2, space="PSUM") as pyp,   # 2 banks
            tc.tile_pool(name="po", bufs=2, space="PSUM") as pop,   # 2 banks
        ):
            # ---- w half 1 first (gates the first QKV), then the small
            # consts; x stream starts in parallel via the lookahead below --
            eps_sb = consts.tile([128, 1], f32, tag="eps")
            nc.gpsimd.memset(eps_sb[:, :], EPS)
            w_lo = consts.tile([128, 4, 384], bf16, tag="wlo")
            nc.sync.dma_start(
                w_lo[:, :, :],
                wp[:, 0:1536].rearrange("p (dc e) -> p dc e", dc=4))
            tri_sb = consts.tile([128, 128], bf16, tag="tri")
            nc.gpsimd.dma_start(tri_sb[:, :], tri)
            id_sb = consts.tile([128, 128], bf16, tag="iden")
            nc.gpsimd.dma_start(id_sb[:, :], iden)
            sel_sb = consts.tile([64, 128], bf16, tag="sel")
            nc.gpsimd.dma_start(sel_sb[:, :], sel)

            q_nat = persist.tile([128, NTT, 128], bf16, tag="qnat")
            k_nat = persist.tile([128, NTT, 128], bf16, tag="knat")
            # rider layout: v0 | ones | v1 | ones (denominator rides as
            # redundant ones columns in the M=96 PV matmul)
            v_sb = persist.tile([128, NTT, 192], bf16, tag="v")
            QT = persist.tile([128, T], bf16, tag="QT")
            KT = persist.tile([128, T], bf16, tag="KT")
            yT = persist.tile([128, T], bf16, tag="yT")

            xts = {}
            dma_done = [0]

            def issue_dma(tt):
                if tt >= NTT or tt < dma_done[0]:
                    return
                dma_done[0] = tt + 1
                xt = xstream.tile([128, 8, 128], bf16, tag="xt",
                                  name=f"xt{tt}")
                nc.sync.dma_start(
                    xt[:, 0:4, :],
                    xt4[:, 1024 * tt:1024 * tt + 512]
                    .rearrange("p (dc t) -> p dc t", dc=4))
                nc.sync.dma_start(
                    xt[:, 4:8, :],
                    xt4[:, 1024 * tt + 512:1024 * tt + 1024]
                    .rearrange("p (dc t) -> p dc t", dc=4))
                xts[tt] = xt
                nc.gpsimd.dma_start(v_sb[:, tt, :],
                                    vep[:, 192 * tt:192 * tt + 192])

            issue_dma(0)
            w_hi = consts.tile([128, 4, 384], bf16, tag="whi")
            nc.sync.dma_start(
                w_hi[:, :, :],
                wp[:, 1536:3072].rearrange("p (dc e) -> p dc e", dc=4))
            issue_dma(1)
            issue_dma(2)
            issue_dma(3)
            # remaining consts after the first x tiles, on the Pool and ACT
            # descriptor rings so they don't serialize behind the x stream
            cos_sb = consts.tile([128, NTT, 16], bf16, tag="cos")
            nc.gpsimd.dma_start(cos_sb[:, :, :],
                                cosp.rearrange("p (tt i) -> p tt i", i=16))
            sin_sb = consts.tile([128, NTT, 16], bf16, tag="sin")
            nc.gpsimd.dma_start(sin_sb[:, :, :],
                                sinp.rearrange("p (tt i) -> p tt i", i=16))
            wc_sb = consts.tile([128, DIM], bf16, tag="wc")
            nc.scalar.dma_start(wc_sb[:, :], wcT)

            # ---- work item generators ----
            def w1(tt):
                """QKV projection of tile tt + eviction. Early tiles evict
                q/k via the ACT engine (idle until the exp stream ramps);
                late tiles use DVE (which has slack once prep ends)."""
                def f():
                    issue_dma(tt + 4)
                    xt = xts.pop(tt)
                    ps = scr.tile([128, 1024], f32, tag="sc", name=f"ps1_{tt}")
                    for dc in range(8):
                        wt = w_lo if dc < 4 else w_hi
                        nc.tensor.matmul(ps[:, 0:384], xt[:, dc, :],
                                         wt[:, dc % 4, :],
                                         start=(dc == 0), stop=(dc == 7))
                    cp = nc.scalar.copy if tt < 12 else nc.vector.tensor_copy
                    cp(q_nat[:, tt, :], ps[:, 0:128])
                    cp(k_nat[:, tt, :], ps[:, 128:256])
                    nc.vector.tensor_add(v_sb[:, tt, 0:64], ps[:, 256:320],
                                         v_sb[:, tt, 0:64])
                    nc.vector.tensor_add(v_sb[:, tt, 96:160], ps[:, 320:384],
                                         v_sb[:, tt, 96:160])
                return f

            def w2a(g):
                """sum-of-squares + rsqrt for tiles 4g..4g+3.
                rsqrt(ms+eps) = exp(-0.5*ln(ms+eps)): Ln+Exp share an ACT
                table so there are no table reloads against the softmax
                exp stream."""
                def f():
                    gsl = slice(G * g, G * g + G)
                    ssA = small.tile([128, 2, G, 2], f32, tag="ssA",
                                     name=f"ss{g}")
                    for qki, nat in enumerate((q_nat, k_nat)):
                        xg = nat[:, gsl, :]
                        sq = tmp.tile([128, G * 128], bf16, tag="sq")
                        nc.vector.tensor_mul(sq[:, :], xg, xg)
                        nc.vector.reduce_sum(
                            ssA[:, qki, :, :],
                            sq[:, :].rearrange("p (a h d) -> p a h d",
                                               a=G, h=2),
                            axis=mybir.AxisListType.X)
                    lnt = small.tile([128, 2, G, 2], f32, tag="lnt",
                                     name=f"ln{g}")
                    nc.scalar.activation(lnt[:, :, :, :], ssA[:, :, :, :],
                                         AF.Ln, bias=eps_sb[:, :],
                                         scale=1.0 / HD)
                    rinv = small.tile([128, 2, G, 2], f32, tag="rinv",
                                      name=f"ri{g}")
                    nc.scalar.activation(rinv[:, :, :, :], lnt[:, :, :, :],
                                         AF.Exp, scale=-0.5)
                    rinvb = small.tile([128, 2, G, 2], bf16, tag="rinvb",
                                       name=f"rb{g}")
                    nc.vector.tensor_copy(rinvb[:, :, :, :],
                                          rinv[:, :, :, :])
                    return rinvb
                holder = {}

                def run():
                    holder['rinvb'] = f()
                run.holder = holder
                return run

            def w2b(g, w2a_item, qki):
                """normalize + rope one side (q or k) of group g."""
                def f():
                    rinvb = w2a_item.holder['rinvb']
                    gsl = slice(G * g, G * g + G)
                    nat = (q_nat, k_nat)[qki]
                    xg4 = nat[:, gsl, :].rearrange(
                        "p a (h d) -> p a h d", h=2)
                    nc.vector.tensor_mul(
                        xg4, xg4,
                        rinvb[:, qki, :, :]
                        .rearrange("p a h -> p a h ()")
                        .broadcast_to((128, G, 2, HD)))
                    # rope on pairs (d, d+32), d in [0,16)
                    x1 = xg4[:, :, :, 0:16]
                    x2 = xg4[:, :, :, 32:48]
                    cg = (cos_sb[:, gsl, :].rearrange("p a i -> p a () i")
                          .broadcast_to((128, G, 2, 16)))
                    sg = (sin_sb[:, gsl, :].rearrange("p a i -> p a () i")
                          .broadcast_to((128, G, 2, 16)))
                    t1 = tmp.tile([128, G, 2, 16], bf16, tag="t1")
                    t2 = tmp.tile([128, G, 2, 16], bf16, tag="t2")
                    t3 = tmp.tile([128, G, 2, 16], bf16, tag="t3")
                    t4 = tmp.tile([128, G, 2, 16], bf16, tag="t4")
                    nc.vector.tensor_mul(t1[:, :, :, :], x1, cg)
                    nc.vector.tensor_mul(t2[:, :, :, :], x2, sg)
                    nc.vector.tensor_mul(t3[:, :, :, :], x1, sg)
                    nc.vector.tensor_mul(t4[:, :, :, :], x2, cg)
                    nc.vector.tensor_add(x1, t1[:, :, :, :],
                                         t2[:, :, :, :])
                    nc.vector.tensor_sub(x2, t4[:, :, :, :],
                                         t3[:, :, :, :])
                return f

            def w3(tt):
                """transpose tile tt into QT/KT via a po-ring slot (bf16
                bitcast view; q and k land in the same bank, so the k
                transpose must not re-clear it -> separate slots)."""
                def f():
                    for ni, (nat, dstT) in enumerate(((q_nat, QT),
                                                      (k_nat, KT))):
                        slot = pop.tile([128, 512], f32, tag="po",
                                        name=f"tp{tt}_{ni}")
                        sb = slot[:, 0:64].bitcast(bf16)
                        nc.tensor.transpose(sb, nat[:, tt, :], id_sb[:, :])
                        cp = (nc.scalar.copy if tt < 12
                              else nc.vector.tensor_copy)
                        cp(dstT[:, 128 * tt:128 * tt + 128], sb)
                return f

            # ---- attention emission ----
            def emit_y(py, pts, ntc, head_major=False):
                """PV accumulation with lhsT = [V_h(64) | ones(32)]: M=96,
                the ones columns accumulate the softmax denominator free.
                head_major orders the final flush so head 0 completes while
                head 1's PVs still stream (the den chain starts earlier)."""
                order = ([(h, c, ql, pt) for h in range(2)
                          for (c, ql, pt) in pts] if head_major else
                         [(h, c, ql, pt) for (c, ql, pt) in pts
                          for h in range(2)])
                for (h, c, ql, pt) in order:
                    nc.tensor.matmul(py[h][:, ql:512],
                                     v_sb[:, c, 96 * h:96 * h + 96],
                                     pt[:, h, ql:512],
                                     start=(c == 0), stop=(c == ntc - 1))

            def cproj(tt, qb, holder):
                def f():
                    ts2 = slice(128 * tt, 128 * tt + 128)
                    # per-tile softmax normalize of yT: lets tile tt's
                    # c_proj start as soon as its own 128 columns are
                    # normalized (pipelines the last q-block's tail)
                    lt = 128 * (tt - 4 * qb)
                    for h in range(2):
                        nc.vector.tensor_mul(
                            yT[64 * h:64 * h + 64, ts2],
                            holder['ysb2'][64 * h:64 * h + 64,
                                           lt:lt + 128],
                            holder['rrs'][64 * h:64 * h + 64,
                                          lt:lt + 128])
                    for half in range(2):
                        po = pop.tile([128, 512], f32, tag="po",
                                      name=f"po{tt}_{half}")
                        nc.tensor.matmul(
                            po[:, :], yT[:, ts2],
                            wc_sb[:, 512 * half:512 * half + 512],
                            start=True, stop=True)
                        ob = obpool.tile([128, 512], bf16, tag="ob",
                                         name=f"ob{tt}_{half}")
                        # early tiles land while the exp stream is sparse:
                        # ACT absorbs those casts, DVE keeps the late ones
                        if tt < 12:
                            nc.scalar.copy(ob[:, :], po[:, :])
                        else:
                            nc.vector.tensor_copy(ob[:, :], po[:, :])
                        # all output DMAs on the Sync ring: the Pool
                        # ring's context-exit drain runs first and would
                        # serialize the teardown behind them
                        nc.sync.dma_start(
                            outp[ts2, 512 * half:512 * half + 512],
                            ob[:, :])
                return f

            # ---- prologue: prep groups 0 and 1 (the per-group chain
            # DMA->QKV->evict->norm->rope->transpose has ~10us latency;
            # early q-blocks are shorter than that, so prep runs 2 groups
            # ahead of the q-block that consumes it) ----
            a0 = w2a(0)
            a1 = w2a(1)
            prologue = [w1(0), w1(1), w1(2), w1(3),
                        a0, w1(4), w1(5), w2b(0, a0, 0), w2b(0, a0, 1),
                        w1(6), w1(7), w3(0), w3(1),
                        a1, w3(2), w3(3), w2b(1, a1, 0), w2b(1, a1, 1),
                        w3(4), w3(5), w3(6), w3(7)]
            for f in prologue:
                f()

            pending = []  # deferred finalize + next-group prep closures

            for qb in range(NQB):
                q0 = 512 * qb
                ntc = 4 * (qb + 1)
                npairs = ntc // 2
                # queue next group's prep behind any finalize items already
                # pending from qb-1, interleaved prep-first
                if qb < NQB - 2:
                    g = qb + 2
                    a = w2a(g)
                    prep = [w1(G * g), w1(G * g + 1), w1(G * g + 2),
                            w1(G * g + 3), a, w2b(g, a, 0), w2b(g, a, 1),
                            w3(G * g), w3(G * g + 1), w3(G * g + 2),
                            w3(G * g + 3)]
                    fins = pending
                    items = []
                    while prep or fins:
                        if prep:
                            items.append(prep.pop(0))
                        if prep:
                            items.append(prep.pop(0))
                        if fins:
                            items.append(fins.pop(0))
                    pending = items

                py = [pyp.tile([96, 512], f32, tag="py",
                               name=f"py{qb}_{h}") for h in range(2)]
                stage = []  # software pipeline: y lags scores by 3 pairs
                total = len(pending)
                for pi in range(npairs):
                    pts = []
                    schunks = []
                    # both chunks' score matmuls first (contiguous PE
                    # burst), then both exps: fewer PE pipeline breaks
                    for c in (2 * pi, 2 * pi + 1):
                        p = c - 4 * qb
                        ql = max(0, 128 * p)
                        ts = slice(128 * c, 128 * c + 128)
                        s = scr.tile([128, 1024], f32, tag="sc",
                                     name=f"s{qb}_{c}")
                        for h in range(2):
                            hp = slice(64 * h, 64 * h + 64)
                            nc.tensor.matmul(
                                s[:, 512 * h + ql:512 * h + 512],
                                KT[hp, ts], QT[hp, q0 + ql:q0 + 512],
                                start=True, stop=True,
                                tile_position=(64 * h, 0))
                        schunks.append((c, p, ql, s))
                    for (c, p, ql, s) in schunks:
                        pt = ptpool.tile([128, 2, 512], bf16, tag="pt",
                                         name=f"ptile{qb}_{c}")
                        nc.scalar.activation(
                            pt[:, :, ql:512],
                            s[:, :].rearrange("p (h n) -> p h n",
                                              h=2)[:, :, ql:512],
                            AF.Exp, scale=SCALE)
                        if p >= 0:
                            # causal mask on the diagonal chunk: Pool engine
                            # (latency hidden by the y-stage pipeline)
                            nc.gpsimd.tensor_mul(
                                pt[:, :, ql:ql + 128],
                                pt[:, :, ql:ql + 128],
                                tri_sb[:, :].rearrange("p x -> p () x")
                                .broadcast_to((128, 2, 128)))
                        pts.append((c, ql, pt))
                    stage.append(pts)
                    if len(stage) == 5:
                        emit_y(py, stage.pop(0), ntc)
                    # drain queued prep/finalize proportionally
                    want = (total * (pi + 1)) // npairs
                    while total - len(pending) < want:
                        pending.pop(0)()
                while len(stage) > 1:
                    emit_y(py, stage.pop(0), ntc)
                final_pts = stage.pop(0)
                while pending:
                    pending.pop(0)()

                # ---- final PVs head-major + per-head early py eviction:
                # head h's ysb copy + den transpose start while head h+1's
                # PVs still stream (shortens the serial finalize chain,
                # which is the whole tail on the last q-block) ----
                # ysb2: y rows to SBUF bf16 with heads at rows 64h so the
                # later mult's SBUF operands share base partitions.
                # den: rider rows 64:96 hold den replicated x32; transpose
                # so q spreads across lanes, reciprocal 16 els/lane,
                # transpose back; den2 rows {0,32} = 1/den per head.
                ysb2 = ysbpool.tile([128, 512], bf16, tag="ysb",
                                    name=f"ysb{qb}")
                xss = []
                for h in range(2):
                    for (c, ql, pt) in final_pts:
                        nc.tensor.matmul(py[h][:, ql:512],
                                         v_sb[:, c, 96 * h:96 * h + 96],
                                         pt[:, h, ql:512],
                                         start=(c == 0), stop=(c == ntc - 1))
                    nc.vector.tensor_copy(ysb2[64 * h:64 * h + 64, :],
                                          py[h][0:64, :])
                    if qb <= 4 or qb == NQB - 1:
                        # mid phase + last block: evict the den rider via a
                        # cheap DVE copy (releases py); the 1/den =
                        # exp(-ln(den)) runs on ACT inside fin_norm, where
                        # its input is long-ready so it can't head-of-line-
                        # block the exp stream (for the last block the exp
                        # stream is already finished)
                        xs = small.tile([32, 512], f32, tag="xs",
                                        name=f"xs{qb}_{h}")
                        nc.vector.tensor_copy(xs[0:32, :], py[h][64:96, :])
                    else:
                        # late phase: ACT is exp-saturated, keep the DVE
                        # stream-transpose reciprocal trick
                        xs = small.tile([32, 512], f32, tag="xs",
                                        name=f"xs{qb}_{h}")
                        nc.vector.transpose(xs[0:32, :], py[h][64:96, :])
                    xss.append(xs)

                def fin_norm(qb=qb, q0=q0, ysb2=ysb2, xss=xss):
                    rdenb = small.tile([64, 512], bf16, tag="rdenb",
                                       name=f"rb{qb}")
                    if qb <= 4 or qb == NQB - 1:
                        lden = small.tile([64, 512], f32, tag="den2",
                                          name=f"ld{qb}")
                        for h in range(2):
                            nc.scalar.activation(lden[32 * h:32 * h + 32, :],
                                                 xss[h][0:32, :], AF.Ln)
                            nc.scalar.activation(
                                rdenb[32 * h:32 * h + 32, :],
                                lden[32 * h:32 * h + 32, :],
                                AF.Exp, scale=-1.0)
                    else:
                        den2 = small.tile([64, 512], f32, tag="den2",
                                          name=f"d2{qb}")
                        for h in range(2):
                            xv = xss[h][0:32, :].rearrange(
                                "p (j c) -> p j c", c=32)
                            nc.vector.reciprocal(xv[:, :, 0:1],
                                                 xv[:, :, 0:1])
                            nc.vector.transpose(den2[32 * h:32 * h + 32, :],
                                                xss[h][0:32, :])
                        nc.vector.tensor_copy(rdenb[0:33, :], den2[0:33, :])
                    if debug:
                        nc.sync.dma_start(dDEN[:, q0:q0 + 512],
                                          rdenb[0:64, :])
                    rp = pop.tile([128, 512], f32, tag="po",
                                  name=f"rp{qb}")
                    nc.tensor.matmul(rp[:, :], sel_sb[0:33, :],
                                     rdenb[0:33, :], start=True, stop=True)
                    rrs = tmp.tile([128, 512], bf16, tag="rrs",
                                   name=f"rrs{qb}")
                    nc.vector.tensor_copy(rrs[:, :], rp[:, :])
                    fin_norm.holder['rrs'] = rrs

                fin_norm.holder = {'ysb2': ysb2}
                pending.append(fin_norm)
                for tt in range(4 * qb, 4 * qb + 4):
                    pending.append(cproj(tt, qb, fin_norm.holder))
            while pending:
                pending.pop(0)()
            if debug:
                nc.sync.dma_start(dQT[:, :], QT[:, :])
                nc.sync.dma_start(dKT[:, :], KT[:, :])
                nc.sync.dma_start(
                    dV[:, :], v_sb[:, :, :].rearrange("p tt d -> p (tt d)"))
                nc.sync.dma_start(dYT[:, :], yT[:, :])
    _cap_matmul_waits(nc)
    return nc


def _cap_matmul_waits(nc, limit=1):
    """walrus supports few (often one) sync-wait slots per lowered
    instruction; move excess waits onto same-engine nops inserted just
    before, so the sequencer blocks identically but each instruction
    carries at most `limit` waits."""
    import bass_rust
    from concourse import mybir

    eng = {
        mybir.EngineType.PE: nc.tensor,
        mybir.EngineType.DVE: nc.vector,
        mybir.EngineType.Activation: nc.scalar,
        mybir.EngineType.Pool: nc.gpsimd,
        mybir.EngineType.SP: nc.sync,
    }

    def make_nop(e):
        eng[e].nop()
        fn = nc.m.functions[0]
        for obb in fn.blocks:
            if (obb.instructions
                    and type(obb.instructions[-1]).__name__ == 'InstNoOp'):
                return obb.instructions.pop()
        raise AssertionError('nop not found')

    fn = nc.m.functions[0]
    for bb in fn.blocks:
        il = bb.instructions
        i = 0
        while i < len(il):
            inst = il[i]
            si = inst.sync_info
            if (si and si.on_wait and len(si.on_wait) > limit
                    and inst.engine in eng
                    and type(inst).__name__ != 'InstNoOp'):
                waits = list(si.on_wait)
                keep, excess = waits[-limit:], waits[:-limit]
                for w in excess:
                    nop = make_nop(inst.engine)
                    nop.sync_info = bass_rust.SyncInfo(on_wait=[w],
                                                       on_update=[])
                    il.insert(i, nop)
                    i += 1
                inst.sync_info = bass_rust.SyncInfo(
                    on_wait=keep, on_update=list(si.on_update))
            i += 1


def _host_prep(x, ve, qkv_w, lambdas, c_proj_w, T):
    import ml_dtypes
    bf = ml_dtypes.bfloat16
    NTT = T // 128
    xf = x.reshape(T, DIM)
    # xt4[p, tt, dc, ti] = x[128*tt+ti, 128*dc+p]
    xt4 = np.ascontiguousarray(
        xf.reshape(NTT, 128, 8, 128).transpose(3, 0, 2, 1)
        .reshape(128, NTT * 1024).astype(bf))
    af = (1.0 / 1024.0) ** np.linspace(0.0, 1.0, HD // 4, dtype=np.float32)
    theta = np.arange(T, dtype=np.float32)[:, None] * af[None, :]
    # cosp[p, tt, i] = cos[128*tt+p, i]
    cosp = np.ascontiguousarray(
        np.cos(theta).reshape(NTT, 128, 16).transpose(1, 0, 2)
        .reshape(128, NTT * 16).astype(bf))
    sinp = np.ascontiguousarray(
        np.sin(theta).reshape(NTT, 128, 16).transpose(1, 0, 2)
        .reshape(128, NTT * 16).astype(bf))
    tri = np.ascontiguousarray(np.triu(np.ones((128, 128), np.float32)).astype(bf))
    iden = np.ascontiguousarray(np.eye(128, dtype=np.float32).astype(bf))
    lam = np.asarray(lambdas, np.float32)
    vef = ve.reshape(T, DIM)
    selm = np.zeros((64, 128), np.float32)
    selm[0, 0:64] = 1.0
    selm[32, 64:128] = 1.0
    selm = np.ascontiguousarray(selm.astype(bf))
    in_maps = []
    for c in range(NC_CORES):
        sl = slice(128 * c, 128 * c + 128)
        wq = qkv_w[0][sl]
        wk = qkv_w[1][sl]
        wv = qkv_w[2][sl] * lam[0]
        wTl = np.concatenate([wq, wk, wv], 0).T  # [1024, 384]
        # wp[p, dc, e] = wTl[128*dc+p, e]
        wpl = np.ascontiguousarray(
            wTl.reshape(8, 128, 384).transpose(1, 0, 2)
            .reshape(128, 8 * 384).astype(bf))
        ve_l = np.ones((T, 192), np.float32)
        ve_l[:, 0:64] = vef[:, sl.start:sl.start + 64] * lam[1]
        ve_l[:, 96:160] = vef[:, sl.start + 64:sl.stop] * lam[1]
        # vep[p, tt, d] = ve_l[128*tt+p, d]
        vepl = np.ascontiguousarray(
            ve_l.reshape(NTT, 128, 192).transpose(1, 0, 2)
            .reshape(128, NTT * 192).astype(bf))
        wcTl = np.ascontiguousarray(c_proj_w[:, sl].T.astype(bf))
        in_maps.append(dict(xt4=xt4, wp=wpl, vep=vepl, cosp=cosp, sinp=sinp,
                            tri=tri, iden=iden, wcT=wcTl, sel=selm))
    return in_maps


LAST_RESULTS = None


def kernel(x, ve, qkv_w, lambdas, c_proj_w):
    import sys
    if '/opt/trn_rl_repo' not in sys.path:
        sys.path.insert(0, '/opt/trn_rl_repo')
    from concourse.bass_utils import run_bass_kernel_spmd

    x = np.asarray(x, np.float32)
    T = x.shape[1]
    in_maps = _host_prep(np.asarray(x, np.float32), np.asarray(ve, np.float32),
                         np.asarray(qkv_w, np.float32),
                         np.asarray(lambdas, np.float32),
                         np.asarray(c_proj_w, np.float32), T)
    nc = _build(T)
    res = run_bass_kernel_spmd(nc, in_maps, core_ids=list(range(NC_CORES)))
    global LAST_RESULTS
    LAST_RESULTS = res
    out = np.zeros((T, DIM), np.float32)
    for rmap in res.results:
        out += rmap["outp"].astype(np.float32)
    return out.reshape(1, T, DIM)
